# revision 12
# baseline (speedup 1.0000x reference)
"""Trainium2 Bass kernel for nn_CCM: per-pixel complex 3x3 mask stencil.

Computation (per batch b):
  H_c = m[c] + v1*m[9+c] + v2*m[18+c],  v1/v2 = -1/2 +- i*sqrt(3)/2, c in 0..8
  out(t,f) = sum_c H_c(t,f) * xpad(t + c//3, f + c%3)   (complex)
with xpad zero-padded by 2 rows at the top (causal time) and 1 col each side.

Sharding: pure data-parallel over B=8 across the 8 NeuronCores.

v4 design:
  - Host precomputes the 27 mask channels (hre_c, him_c, hsum_c=hre+him)
    in fp32, casts once to bf16 -> mask DMA is 27 contiguous channels.
  - Karatsuba complex product per tap:
      k1 = hre*xr, k2 = him*xi, k3 = hsum*(xr+xi)
      re += k1 - k2 ; im += k3 - k1 - k2
    -> 27 products + 24 tree-adds + 3 merges, ALL on VectorE at bf16
    2x mode.  GpSimd does no elementwise work (SBUF-port contention
    with DVE measured ~3.8x slowdown).
  - x planes (xr, xi, s=xr+xi) are host-padded/replicated for the 3 row
    shifts and duplicated at two byte parities so every product slice is
    4B-aligned (keeps DVE 2x mode; nn=1 taps read the odd-parity copy).
  - All DMA is HWDGE (sync+scalar queues), one contiguous run per
    partition per transfer (>=4KB descriptors).
  - Tree (pairwise) accumulation keeps bf16 rounding error at 1.2e-2
    scale-relative (serial chains measured 1.7e-2, budget 2e-2).
"""

import sys

import numpy as np

sys.path.insert(0, "/opt/trn_rl_repo")

B, T, F = 8, 1000, 257
TP = 125          # partitions; time row t = kk*TP + p
KK = 8            # time chunks
FB = 258          # padded op width (even element count for bf16 2x mode)
XE = 260          # even-parity x row width (covers col shifts 0 and 2)
XO = 258          # odd-parity x row width (col shift 1, pre-shifted)
SQ3_2 = float(np.sqrt(3.0) / 2.0)

_prog_cache = {}


def _build_program():
    import concourse.tile as tile
    from concourse import bacc, mybir

    bf16 = mybir.dt.bfloat16

    nc = bacc.Bacc()
    # (hre, him, hsum) per tap, one contiguous 12.4KB run per partition
    mk_d = nc.declare_dram_parameter("mk", [9, TP, 3, KK, FB], bf16,
                                     isOutput=False)
    # x planes stacked per (rowshift rep, parity): [rep, p, plane, kk, w]
    xe_d = nc.declare_dram_parameter("xe", [3, TP, 3, KK, XE], bf16,
                                     isOutput=False)
    xo_d = nc.declare_dram_parameter("xo", [3, TP, 3, KK, XO], bf16,
                                     isOutput=False)
    out_d = nc.declare_dram_parameter("out", [TP, 2, KK, FB], bf16,
                                      isOutput=True)

    with tile.TileContext(nc) as tc:
        from contextlib import ExitStack

        with ExitStack() as ctx:
            xpool = ctx.enter_context(tc.tile_pool(name="xpool", bufs=1))
            mpool = ctx.enter_context(tc.tile_pool(name="mpool", bufs=3))
            wpool = ctx.enter_context(tc.tile_pool(name="wpool", bufs=2))
            ppool = ctx.enter_context(tc.tile_pool(name="ppool", bufs=2))
            opool = ctx.enter_context(tc.tile_pool(name="opool", bufs=1))

            dma_engines = [nc.sync, nc.scalar]
            ndma = [0]

            def dma(out, in_):
                eng = dma_engines[ndma[0] % 2]
                ndma[0] += 1
                eng.dma_start(out=out, in_=in_)

            # ---- DMA issue plan: x plane stacks + mask triples, ordered
            # so tap order 0,2,1,3,5,4,6,8,7 can start ASAP.  A mask DMA
            # that reuses a ring slot only ever waits on a tap whose own
            # inputs were issued earlier on both queues (no cycles).
            xe_t = {}
            xo_t = {}
            mk_t = {}

            def load_xe(rep):
                t = xpool.tile([TP, 3, KK, XE], bf16, tag=f"xe{rep}",
                               name=f"xe{rep}")
                dma(t, xe_d[rep])
                xe_t[rep] = t

            def load_xo(rep):
                t = xpool.tile([TP, 3, KK, XO], bf16, tag=f"xo{rep}",
                               name=f"xo{rep}")
                dma(t, xo_d[rep])
                xo_t[rep] = t

            def load_mk(c):
                t = mpool.tile([TP, 3, KK, FB], bf16, tag="mk",
                               name=f"mk{c}")
                dma(t, mk_d[c])
                mk_t[c] = t

            load_xe(0)
            load_mk(0)
            load_mk(2)
            load_xo(0)
            load_mk(1)
            load_xe(1)
            load_mk(3)
            load_mk(5)
            load_xo(1)
            load_mk(4)
            load_xe(2)
            load_mk(6)
            load_mk(8)
            load_xo(2)
            load_mk(7)

            # ---- Compute: one stacked product op per tap covering the
            # three Karatsuba components (k1|k2|k3 on the plane axis),
            # then a balanced pairwise tree matching tap arrival order:
            #   (((0+2)+(1+3)) + ((5+4)+(6+8))) + 7
            nt = [0]

            def prod_op(c):
                mm, nn = divmod(c, 3)
                if nn == 1:
                    xs = xo_t[mm][:, :, :, 0:FB]
                else:
                    xs = xe_t[mm][:, :, :, nn:nn + FB]
                p = ppool.tile([TP, 3, KK, FB], bf16, tag="prod",
                               name=f"p{c}")
                nc.vector.tensor_mul(p, mk_t[c], xs)
                return p

            def add(tag, a, b, name, bufs=None):
                t = wpool.tile([TP, 3, KK, FB], bf16, tag=tag, name=name,
                               bufs=bufs)
                nc.vector.tensor_add(t, a, b)
                return t

            p0 = prod_op(0)
            p2 = prod_op(2)
            l1a = add("L1", p0, p2, "l1a")
            p1 = prod_op(1)
            p3 = prod_op(3)
            l1b = add("L1", p1, p3, "l1b")
            l2a = add("L2", l1a, l1b, "l2a")
            p5 = prod_op(5)
            p4 = prod_op(4)
            l1c = add("L1", p5, p4, "l1c")
            p6 = prod_op(6)
            p8 = prod_op(8)
            l1d = add("L1", p6, p8, "l1d")
            l2b = add("L2", l1c, l1d, "l2b")
            l3 = add("L3", l2a, l2b, "l3", bufs=1)
            p7 = prod_op(7)
            fin = add("L1", l3, p7, "fin")
            A = fin[:, 0]
            Bc = fin[:, 1]
            Cc = fin[:, 2]

            out_t = opool.tile([TP, 2, KK, FB], bf16, tag="out")
            nc.vector.tensor_sub(out_t[:, 0], A, Bc)           # re = A - B
            tsum = wpool.tile([TP, 1, KK, FB], bf16, tag="L2", name="tsum")
            nc.vector.tensor_add(tsum[:, 0], A, Bc)
            nc.vector.tensor_sub(out_t[:, 1], Cc, tsum[:, 0])  # im = C-(A+B)

            dma(out_d[:, :, :, :], out_t)

    nc.finalize()
    return nc


def _get_program():
    if "nc" not in _prog_cache:
        _prog_cache["nc"] = _build_program()
    return _prog_cache["nc"]


def _host_prep(m, x):
    import ml_dtypes

    bf = ml_dtypes.bfloat16
    in_maps = []
    for b in range(B):
        mr = m[b].reshape(3, 9, T, F)
        hre = mr[0] - 0.5 * (mr[1] + mr[2])
        him = SQ3_2 * (mr[1] - mr[2])
        hsum = hre + him
        # [3ch, 9, T, F] -> [9, TP, 3, KK, FB]; t = kk*TP + p
        mk = np.zeros((3, 9, KK, TP, FB), np.float32)
        st = np.stack([hre, him, hsum])           # (3, 9, T, F)
        mk[:, :, :, :, :F] = st.reshape(3, 9, KK, TP, F)
        mk = np.ascontiguousarray(mk.transpose(1, 3, 0, 2, 4)).astype(bf)

        xb = x[b]  # (F, T, 2)
        xrp = np.zeros((T + 2, XE + 2), np.float32)
        xip = np.zeros((T + 2, XE + 2), np.float32)
        xrp[2:, 1:F + 1] = xb[:, :, 0].T
        xip[2:, 1:F + 1] = xb[:, :, 1].T
        sp = xrp + xip
        planes = [xrp, xip, sp]
        xe = np.empty((3, TP, 3, KK, XE), np.float32)
        xo = np.empty((3, TP, 3, KK, XO), np.float32)
        for pl in range(3):
            for rep in range(3):
                for kk in range(KK):
                    # rows t = kk*TP + p, padded row index t + rep
                    r0 = kk * TP + rep
                    xe[rep, :, pl, kk, :] = planes[pl][r0:r0 + TP, 0:XE]
                    xo[rep, :, pl, kk, :] = planes[pl][r0:r0 + TP, 1:1 + XO]
        in_maps.append({"mk": mk, "xe": xe.astype(bf), "xo": xo.astype(bf)})
    return in_maps


def _assemble(results):
    out = np.empty((B, F, T, 2), np.float32)
    for b in range(B):
        arr = results[b]["out"].astype(np.float32)  # [TP, 2, KK, FB]
        # t = kk*TP + p
        a = arr[:, :, :, :F].transpose(1, 2, 0, 3)  # [2, KK, TP, F]
        a = a.reshape(2, T, F)                      # [2, T, F]
        out[b, :, :, 0] = a[0].T
        out[b, :, :, 1] = a[1].T
    return out


def kernel(m, x, _trace=False):
    from concourse.bass_utils import run_bass_kernel_spmd

    nc = _get_program()
    in_maps = _host_prep(np.asarray(m), np.asarray(x))
    res = run_bass_kernel_spmd(nc, in_maps, list(range(B)), trace=_trace)
    out = _assemble(res.results)
    if _trace:
        return out, res
    return out


# revision 14
# speedup vs baseline: 1.3818x; 1.3818x over previous
"""Trainium2 Bass kernel for nn_CCM: per-pixel complex 3x3 mask stencil.

Computation (per batch b):
  H_c = m[c] + v1*m[9+c] + v2*m[18+c],  v1/v2 = -1/2 +- i*sqrt(3)/2, c in 0..8
  out(t,f) = sum_c H_c(t,f) * xpad(t + c//3, f + c%3)   (complex)
with xpad zero-padded by 2 rows at the top (causal time) and 1 col each side.

Sharding: pure data-parallel over B=8 across the 8 NeuronCores.

v4 design:
  - Host precomputes the 27 mask channels (hre_c, him_c, hsum_c=hre+him)
    in fp32, casts once to bf16 -> mask DMA is 27 contiguous channels.
  - Karatsuba complex product per tap:
      k1 = hre*xr, k2 = him*xi, k3 = hsum*(xr+xi)
      re += k1 - k2 ; im += k3 - k1 - k2
    -> 27 products + 24 tree-adds + 3 merges, ALL on VectorE at bf16
    2x mode.  GpSimd does no elementwise work (SBUF-port contention
    with DVE measured ~3.8x slowdown).
  - x planes (xr, xi, s=xr+xi) are host-padded/replicated for the 3 row
    shifts and duplicated at two byte parities so every product slice is
    4B-aligned (keeps DVE 2x mode; nn=1 taps read the odd-parity copy).
  - All DMA is HWDGE (sync+scalar queues), one contiguous run per
    partition per transfer (>=4KB descriptors).
  - Tree (pairwise) accumulation keeps bf16 rounding error at 1.2e-2
    scale-relative (serial chains measured 1.7e-2, budget 2e-2).
"""

import sys

import numpy as np

sys.path.insert(0, "/opt/trn_rl_repo")

B, T, F = 8, 1000, 257
TP = 125          # partitions; time row t = kk*TP + p
KK = 8            # time chunks
FB = 258          # padded op width (even element count for bf16 2x mode)
XE = 260          # even-parity x row width (covers col shifts 0 and 2)
XO = 258          # odd-parity x row width (col shift 1, pre-shifted)
SQ3_2 = float(np.sqrt(3.0) / 2.0)

_prog_cache = {}


def _build_program():
    import concourse.tile as tile
    from concourse import bacc, mybir

    bf16 = mybir.dt.bfloat16

    nc = bacc.Bacc()
    # (hre, him, hsum) per tap, one contiguous 12.4KB run per partition
    mk_d = nc.declare_dram_parameter("mk", [9, TP, 3, KK, FB], bf16,
                                     isOutput=False)
    # x planes stacked per (rowshift rep, parity): [rep, p, plane, kk, w]
    xe_d = nc.declare_dram_parameter("xe", [3, TP, 3, KK, XE], bf16,
                                     isOutput=False)
    xo_d = nc.declare_dram_parameter("xo", [3, TP, 3, KK, XO], bf16,
                                     isOutput=False)
    out_d = nc.declare_dram_parameter("out", [TP, 2, KK, FB], bf16,
                                      isOutput=True)

    with tile.TileContext(nc) as tc:
        from contextlib import ExitStack

        with ExitStack() as ctx:
            xpool = ctx.enter_context(tc.tile_pool(name="xpool", bufs=1))
            mpool = ctx.enter_context(tc.tile_pool(name="mpool", bufs=4))
            ppool = ctx.enter_context(tc.tile_pool(name="ppool", bufs=1))
            opool = ctx.enter_context(tc.tile_pool(name="opool", bufs=1))

            # All loads go through SWDGE (gpsimd): HWDGE (sync/scalar)
            # queues share only 5 SDMA engines on this system (~130GB/s
            # combined), while one SWDGE DMA spreads across all 16
            # engines (~26GB/s each -> HBM-limited ~358GB/s).
            def dma(out, in_):
                nc.gpsimd.dma_start(out=out, in_=in_)

            # ---- DMA issue plan: x plane stacks + mask triples, ordered
            # so tap order 0,2,1,3,5,4,6,8,7 can start ASAP.  A mask DMA
            # that reuses a ring slot only ever waits on a tap whose own
            # inputs were issued earlier on both queues (no cycles).
            xe_t = {}
            xo_t = {}
            mk_t = {}

            def load_xe(rep):
                t = xpool.tile([TP, 3, KK, XE], bf16, tag=f"xe{rep}",
                               name=f"xe{rep}")
                dma(t, xe_d[rep])
                xe_t[rep] = t

            def load_xo(rep):
                t = xpool.tile([TP, 3, KK, XO], bf16, tag=f"xo{rep}",
                               name=f"xo{rep}")
                dma(t, xo_d[rep])
                xo_t[rep] = t

            def load_mk(c):
                t = mpool.tile([TP, 3, KK, FB], bf16, tag="mk",
                               name=f"mk{c}")
                dma(t, mk_d[c])
                mk_t[c] = t

            load_xe(0)
            load_mk(0)
            load_mk(2)
            load_xo(0)
            load_mk(1)
            load_xe(1)
            load_mk(3)
            load_mk(5)
            load_xo(1)
            load_mk(4)
            load_xe(2)
            load_mk(6)
            load_mk(8)
            load_xo(2)
            load_mk(7)

            # ---- Compute: one stacked product op per tap covering the
            # three Karatsuba components (k1|k2|k3 on the plane axis),
            # then a balanced pairwise tree (in-place accumulation)
            # matching tap arrival order:
            #   (((0+2)+(1+3)) + ((5+4)+(6+8))) + 7
            def prod_op(c, tag, bufs=1):
                mm, nn = divmod(c, 3)
                if nn == 1:
                    xs = xo_t[mm][:, :, :, 0:FB]
                else:
                    xs = xe_t[mm][:, :, :, nn:nn + FB]
                p = ppool.tile([TP, 3, KK, FB], bf16, tag=tag, bufs=bufs,
                               name=f"p{c}")
                nc.vector.tensor_mul(p, mk_t[c], xs)
                return p

            acc0 = prod_op(0, "acc0")                 # p0
            f0 = prod_op(2, "feed", bufs=2)
            nc.vector.tensor_add(acc0, acc0, f0)      # 0+2
            t0 = prod_op(1, "t0")
            f1 = prod_op(3, "feed", bufs=2)
            nc.vector.tensor_add(t0, t0, f1)          # 1+3
            nc.vector.tensor_add(acc0, acc0, t0)      # (0+2)+(1+3)
            acc1 = prod_op(5, "acc1")
            f2 = prod_op(4, "feed", bufs=2)
            nc.vector.tensor_add(acc1, acc1, f2)      # 5+4
            t1 = prod_op(6, "t1")
            f3 = prod_op(8, "feed", bufs=2)
            nc.vector.tensor_add(t1, t1, f3)          # 6+8
            nc.vector.tensor_add(acc1, acc1, t1)      # (5+4)+(6+8)
            nc.vector.tensor_add(acc0, acc0, acc1)    # left + right
            f4 = prod_op(7, "feed", bufs=2)
            nc.vector.tensor_add(acc0, acc0, f4)      # ... + 7
            A = acc0[:, 0]
            Bc = acc0[:, 1]
            Cc = acc0[:, 2]

            out_t = opool.tile([TP, 2, KK, FB], bf16, tag="out")
            nc.vector.tensor_sub(out_t[:, 0], A, Bc)           # re = A - B
            tsum = ppool.tile([TP, 1, KK, FB], bf16, tag="t0", name="tsum")
            nc.vector.tensor_add(tsum[:, 0], A, Bc)
            nc.vector.tensor_sub(out_t[:, 1], Cc, tsum[:, 0])  # im = C-(A+B)

            dma(out_d[:, :, :, :], out_t)

    nc.finalize()
    return nc


def _get_program():
    if "nc" not in _prog_cache:
        _prog_cache["nc"] = _build_program()
    return _prog_cache["nc"]


def _host_prep(m, x):
    import ml_dtypes

    bf = ml_dtypes.bfloat16
    in_maps = []
    for b in range(B):
        mr = m[b].reshape(3, 9, T, F)
        hre = mr[0] - 0.5 * (mr[1] + mr[2])
        him = SQ3_2 * (mr[1] - mr[2])
        hsum = hre + him
        # [3ch, 9, T, F] -> [9, TP, 3, KK, FB]; t = kk*TP + p
        mk = np.zeros((3, 9, KK, TP, FB), np.float32)
        st = np.stack([hre, him, hsum])           # (3, 9, T, F)
        mk[:, :, :, :, :F] = st.reshape(3, 9, KK, TP, F)
        mk = np.ascontiguousarray(mk.transpose(1, 3, 0, 2, 4)).astype(bf)

        xb = x[b]  # (F, T, 2)
        xrp = np.zeros((T + 2, XE + 2), np.float32)
        xip = np.zeros((T + 2, XE + 2), np.float32)
        xrp[2:, 1:F + 1] = xb[:, :, 0].T
        xip[2:, 1:F + 1] = xb[:, :, 1].T
        sp = xrp + xip
        planes = [xrp, xip, sp]
        xe = np.empty((3, TP, 3, KK, XE), np.float32)
        xo = np.empty((3, TP, 3, KK, XO), np.float32)
        for pl in range(3):
            for rep in range(3):
                for kk in range(KK):
                    # rows t = kk*TP + p, padded row index t + rep
                    r0 = kk * TP + rep
                    xe[rep, :, pl, kk, :] = planes[pl][r0:r0 + TP, 0:XE]
                    xo[rep, :, pl, kk, :] = planes[pl][r0:r0 + TP, 1:1 + XO]
        in_maps.append({"mk": mk, "xe": xe.astype(bf), "xo": xo.astype(bf)})
    return in_maps


def _assemble(results):
    out = np.empty((B, F, T, 2), np.float32)
    for b in range(B):
        arr = results[b]["out"].astype(np.float32)  # [TP, 2, KK, FB]
        # t = kk*TP + p
        a = arr[:, :, :, :F].transpose(1, 2, 0, 3)  # [2, KK, TP, F]
        a = a.reshape(2, T, F)                      # [2, T, F]
        out[b, :, :, 0] = a[0].T
        out[b, :, :, 1] = a[1].T
    return out


def kernel(m, x, _trace=False):
    from concourse.bass_utils import run_bass_kernel_spmd

    nc = _get_program()
    in_maps = _host_prep(np.asarray(m), np.asarray(x))
    res = run_bass_kernel_spmd(nc, in_maps, list(range(B)), trace=_trace)
    out = _assemble(res.results)
    if _trace:
        return out, res
    return out
